# revision 20
# baseline (speedup 1.0000x reference)
"""Trainium2 Bass kernel for nn_CausalAttentionPooling.

Math: scores[b,i,j] = x[b,i].q are constant along the softmax axis j, so
softmax over the causal mask yields uniform weights 1/(i+1) on j <= i.
The module is exactly a causal cumulative mean:
    out[b,i,:] = cumsum(x, axis=1)[b,i,:] / (i+1)
(q does not affect the output.)

Sharding: 8 shards = (batch b in 0..3) x (D-half dh in 0..1); each core gets
x[b, :, dh*128:(dh+1)*128] transposed to [128(D), 4096(L)], cast bf16, and
de-interleaved on host into 4 planes xp[r] = xT[:, r::4] of [128, 1024].

Device algorithm (grouped cumsum, G=4):
  u01 = x0+x1 ; u23 = x2+x3                  (bf16 pair adds, DVE 2x mode)
  group scan absorbs the final pair add:
      state = (u01[g] + state) + u23[g]      (fp32 state) -> cumY bf16
  carry[g] = cumY[g-1]  (shifted view of a [128, NG+1] tile, col 0 = 0)
  c-chain: c0 = x0+carry; c1 = c0+x1; c2 = c1+x2
  o_r = c_r * rr_r.  o3 = cumY * rr_3 runs first, split in halves, so
  output DMAs hit the (idle) bus right after the scan; the last mul o2
  is split h/q/q so the final out-DMA is small.  The ~1MB of outputs
  needs ~4us of shared DMA bus: the tail is output-bandwidth-bound.
rr_r = 1/(4g+r+1) replicated across partitions via PE outer products
(idle engine), drained psum->bf16 SBUF by Act (idle engine).

Measured HW facts that shaped this (NTFF profiles, this chip):
  - ~9.5us fixed preamble+postamble (engine barriers, TENSOR_LOADs).
  - DMA aggregate ~262 B/ns per core SHARED across queues; two HW queues
    (SP, Act) only parallelize the ~630ns per-issue cost; completion
    semaphore posts ~0.9us after last byte.  Pool SWDGE queue is slower,
    not faster.  hwdge engines are only SP and Act.
  - DVE tensor_tensor bf16 SBUF runs 2x ((58+FD/2) cyc @0.96GHz); fp32
    or any-PSUM operand drops to 1x; tensor_tensor_scan is ~2.24ns/col
    regardless of src dtype; dependent-op cadence ~dur+215ns.
  - Per-op fixed cost (~130ns) favors FD=1024 ops: G=4 beats G=8.
  - Pool (gpsimd) tensor ops run ~2ns/elem AND stall concurrent DVE ops
    ~3x (SBUF contention) => Pool does only memsets.  Pool scan fails
    to compile.  Act has no tensor_tensor (activation-class ops only).
The spine (last input byte -> u23 -> scan) and the 7-op DVE wave are
engine/data-bound; ~25us total, vs 33.4/28.9us for the scan-everything
baseline and ~9.5us of it framework-fixed.
"""

import numpy as np

B, L, D = 4, 4096, 256
NCORES = 8
P = 128
G = 4
NG = L // G              # 1024 groups

_cache = {}


def _split_waits_bir(bir_bytes):
    """This container's walrus build rejects instructions carrying more than
    one (or for some opcodes, two) sync waits.  Hoist multi-wait sync_info
    onto standalone same-engine EventSemaphore instructions inserted
    immediately before the instruction; program order on the engine's stream
    preserves semantics."""
    import orjson

    d = orjson.loads(bir_bytes)
    n = 0
    for fn in d["functions"]:
        for bb in fn["blocks"]:
            out = []
            for inst in bb["instructions"]:
                si = inst.get("sync_info")
                waits = (si or {}).get("on_wait") or []
                if len(waits) > 1:
                    for w in waits:
                        out.append(
                            {
                                "debug": inst.get("debug"),
                                "engine": inst["engine"],
                                "ins": [],
                                "name": f"I-waitfix-{n}",
                                "opcode": "EventSemaphore",
                                "outs": [],
                                "sync_info": {"on_wait": [w], "on_update": []},
                            }
                        )
                        n += 1
                    si["on_wait"] = []
                out.append(inst)
            bb["instructions"] = out
    return orjson.dumps(d)


def _install_bir_patch():
    if _cache.get("patched"):
        return
    import concourse.bass as bass

    orig = bass.Bass.to_json_bytes

    def patched(self):
        return _split_waits_bir(orig(self))

    bass.Bass.to_json_bytes = patched
    _cache["patched"] = True


def _build_nc():
    import concourse.bass as bass
    import concourse.tile as tile
    from concourse import mybir

    _install_bir_patch()

    f32 = mybir.dt.float32
    bf16 = mybir.dt.bfloat16
    add = mybir.AluOpType.add
    mult = mybir.AluOpType.mult

    nc = bass.Bass()
    xp = [
        nc.declare_dram_parameter(f"xp{r}", [P, NG], bf16, isOutput=False)
        for r in range(G)
    ]
    rrp = nc.declare_dram_parameter("rrp", [1, L], bf16, isOutput=False)
    outs = [
        nc.declare_dram_parameter(f"o{r}", [P, NG], bf16, isOutput=True)
        for r in range(G)
    ]

    with tile.TileContext(nc) as tc:
        with (
            tc.tile_pool(name="sb", bufs=1) as sb,
            tc.tile_pool(name="ps", bufs=1, space="PSUM") as ps,
        ):
            xs = [sb.tile([P, NG], bf16, tag=f"x{r}", name=f"x{r}") for r in range(G)]
            u01 = sb.tile([P, NG], bf16, tag="u01")
            v012 = sb.tile([P, NG], bf16, tag="v012")
            cumY = sb.tile([P, NG + 1], bf16, tag="cumY")
            cs = [sb.tile([P, NG], bf16, tag=f"c{r}", name=f"c{r}") for r in range(3)]
            os_ = [sb.tile([P, NG], bf16, tag=f"os{r}", name=f"os{r}") for r in range(G)]
            rr_sb = sb.tile([P, L], bf16, tag="rrsb")
            rrow_sb = sb.tile([1, L], bf16, tag="rrow")
            ones = sb.tile([1, P], bf16, tag="ones")

            # ---- input DMAs on two HW queues (aggregate BW is shared; the
            # split only parallelizes the ~630ns per-issue cost).
            # All planes on ONE queue in consumption order: a solo queue
            # sustains the full ~262 B/ns and guarantees ordered arrivals
            # x0 -> x1 -> x2 -> x3, so u01/v012 ride the stream and the
            # scan starts at x3's semaphore.  (Queue arbitration under
            # contention is unfair: a second loaded queue starves.)
            nc.sync.dma_start(xs[0][:], xp[0][:])
            nc.sync.dma_start(xs[1][:], xp[1][:])
            nc.sync.dma_start(xs[2][:], xp[2][:])
            nc.sync.dma_start(xs[3][:], xp[3][:])
            nc.scalar.dma_start(rrow_sb[:], rrp[:])

            # ---- Pool: tiny memsets only (Pool tensor work stalls DVE 3x)
            nc.gpsimd.memset(ones[:], 1.0)
            nc.gpsimd.memset(cumY[:, 0:1], 0.0)

            # ---- PE: replicate rr across partitions; Act drains to bf16.
            # Segment 3 first: o3's mul is the first the wave runs.
            PB = 512
            pair_order = [3, 0, 1, 2]   # o3's rr segment first
            for pj in pair_order:
                pban = ps.tile([P, 2 * PB], f32, tag=f"rrp{pj}", name=f"rrp{pj}")
                for k in range(2):
                    j = 2 * pj + k
                    nc.tensor.matmul(
                        pban[:, k * PB : (k + 1) * PB],
                        ones[:],
                        rrow_sb[:, j * PB : (j + 1) * PB],
                        start=True,
                        stop=True,
                    )
                nc.scalar.copy(rr_sb[:, 2 * pj * PB : (2 * pj + 2) * PB], pban[:])

            def rr(r):
                return rr_sb[:, r * NG : (r + 1) * NG]

            # ---- DVE: pair adds (fp32 out: scan runs 2.3ns/col from fp32)
            nc.vector.tensor_tensor(u01[:], xs[0][:], xs[1][:], op=add)
            nc.vector.tensor_tensor(v012[:], u01[:], xs[2][:], op=add)

            # ---- DVE: group scan; consumes the last-arriving plane raw:
            # state = (v012[g] + state) + x3[g]
            nc.vector.tensor_tensor_scan(
                cumY[:, 1 : NG + 1], v012[:], xs[3][:], 0.0, op0=add, op1=add
            )
            carry = cumY[:, 0:NG]

            # ---- wave: c-chain with muls slotted into dependency bubbles
            h3 = NG // 2
            nc.vector.tensor_tensor(
                os_[3][:, 0:h3], cumY[:, 1 : h3 + 1],
                rr_sb[:, 3 * NG : 3 * NG + h3], op=mult,
            )
            nc.sync.dma_start(outs[3][:, 0:h3], os_[3][:, 0:h3])
            nc.vector.tensor_tensor(
                os_[3][:, h3:NG], cumY[:, h3 + 1 : NG + 1],
                rr_sb[:, 3 * NG + h3 : 4 * NG], op=mult,
            )
            nc.sync.dma_start(outs[3][:, h3:NG], os_[3][:, h3:NG])

            nc.vector.tensor_tensor(cs[0][:], xs[0][:], carry, op=add)
            for r in range(1, 3):
                nc.vector.tensor_tensor(os_[r - 1][:], cs[r - 1][:], rr(r - 1), op=mult)
                nc.vector.tensor_tensor(cs[r][:], cs[r - 1][:], xs[r][:], op=add)
                eng = nc.sync if r % 2 == 0 else nc.scalar
                eng.dma_start(outs[r - 1][:], os_[r - 1][:])
            h = NG // 2
            q = NG // 4
            nc.vector.tensor_tensor(os_[2][:, 0:h], cs[2][:, 0:h], rr_sb[:, 2 * NG : 2 * NG + h], op=mult)
            nc.scalar.dma_start(outs[2][:, 0:h], os_[2][:, 0:h])
            nc.vector.tensor_tensor(os_[2][:, h : h + q], cs[2][:, h : h + q], rr_sb[:, 2 * NG + h : 2 * NG + h + q], op=mult)
            nc.sync.dma_start(outs[2][:, h : h + q], os_[2][:, h : h + q])
            nc.vector.tensor_tensor(os_[2][:, h + q : NG], cs[2][:, h + q : NG], rr_sb[:, 2 * NG + h + q : 3 * NG], op=mult)
            nc.scalar.dma_start(outs[2][:, h + q : NG], os_[2][:, h + q : NG])
    return nc


def _get_nc():
    if "nc" not in _cache:
        _cache["nc"] = _build_nc()
    return _cache["nc"]


def _make_in_maps(x):
    import ml_dtypes

    bf16 = ml_dtypes.bfloat16
    idx = np.arange(1, L + 1, dtype=np.float64)
    rr_full = 1.0 / idx
    # plane-major rr row: rrp[0, r*NG + g] = 1/(4g + r + 1)
    rrp = np.empty((1, L), dtype=bf16)
    for r in range(G):
        rrp[0, r * NG : (r + 1) * NG] = rr_full[r::G].astype(bf16)
    in_maps = []
    shards = []
    for c in range(NCORES):
        b, dh = c // 2, c % 2
        shards.append((b, dh))
        xT = x[b, :, dh * P : (dh + 1) * P].T.astype(bf16)  # [128, 4096]
        m = {"rrp": rrp}
        for r in range(G):
            m[f"xp{r}"] = np.ascontiguousarray(xT[:, r::G])
        in_maps.append(m)
    return in_maps, shards


def kernel(x, q):
    from concourse.bass_utils import run_bass_kernel_spmd

    x = np.asarray(x)
    assert x.shape == (B, L, D) and x.dtype == np.float32

    nc = _get_nc()
    in_maps, shards = _make_in_maps(x)
    results = run_bass_kernel_spmd(nc, in_maps, list(range(NCORES))).results

    out = np.empty((B, L, D), dtype=np.float32)
    full = np.empty((P, L), dtype=np.float32)
    for c, (b, dh) in enumerate(shards):
        for r in range(G):
            full[:, r::G] = results[c][f"o{r}"].astype(np.float32)
        out[b, :, dh * P : (dh + 1) * P] = full.T
    return out


# revision 21
# speedup vs baseline: 1.0885x; 1.0885x over previous
"""Trainium2 Bass kernel for nn_CausalAttentionPooling.

Math: scores[b,i,j] = x[b,i].q are constant along the softmax axis j, so
softmax over the causal mask yields uniform weights 1/(i+1) on j <= i.
The module is exactly a causal cumulative mean:
    out[b,i,:] = cumsum(x, axis=1)[b,i,:] / (i+1)
(q does not affect the output.)

Sharding: 8 shards = (batch b in 0..3) x (D-half dh in 0..1); each core gets
x[b, :, dh*128:(dh+1)*128] transposed to [128(D), 4096(L)], cast bf16, and
de-interleaved on host into 4 planes xp[r] = xT[:, r::4] of [128, 1024].

Device algorithm (grouped cumsum, G=4):
  u01 = x0+x1 ; u23 = x2+x3                  (bf16 pair adds, DVE 2x mode)
  group scan absorbs the final pair add:
      state = (u01[g] + state) + u23[g]      (fp32 state) -> cumY bf16
  carry[g] = cumY[g-1]  (shifted view of a [128, NG+1] tile, col 0 = 0)
  c-chain: c0 = x0+carry; c1 = c0+x1; c2 = c1+x2
  o_r = c_r * rr_r.  o3 = cumY * rr_3 runs first, split in halves, so
  output DMAs hit the (idle) bus right after the scan; the last mul o2
  is split h/q/q so the final out-DMA is small.  The ~1MB of outputs
  needs ~4us of shared DMA bus: the tail is output-bandwidth-bound.
rr_r = 1/(4g+r+1) replicated across partitions via PE outer products
(idle engine), drained psum->bf16 SBUF by Act (idle engine).

Measured HW facts that shaped this (NTFF profiles, this chip):
  - ~9.5us fixed preamble+postamble (engine barriers, TENSOR_LOADs).
  - DMA aggregate ~262 B/ns per core SHARED across queues; two HW queues
    (SP, Act) only parallelize the ~630ns per-issue cost; completion
    semaphore posts ~0.9us after last byte.  Pool SWDGE queue is slower,
    not faster.  hwdge engines are only SP and Act.
  - DVE tensor_tensor bf16 SBUF runs 2x ((58+FD/2) cyc @0.96GHz); fp32
    or any-PSUM operand drops to 1x; tensor_tensor_scan is ~2.24ns/col
    regardless of src dtype; dependent-op cadence ~dur+215ns.
  - Per-op fixed cost (~130ns) favors FD=1024 ops: G=4 beats G=8.
  - Pool (gpsimd) tensor ops run ~2ns/elem AND stall concurrent DVE ops
    ~3x (SBUF contention) => Pool does only memsets.  Pool scan fails
    to compile.  Act has no tensor_tensor (activation-class ops only).
The spine (last input byte -> u23 -> scan) and the 7-op DVE wave are
engine/data-bound; ~25us total, vs 33.4/28.9us for the scan-everything
baseline and ~9.5us of it framework-fixed.
"""

import numpy as np

B, L, D = 4, 4096, 256
NCORES = 8
P = 128
G = 4
NG = L // G              # 1024 groups

_cache = {}


def _split_waits_bir(bir_bytes):
    """This container's walrus build rejects instructions carrying more than
    one (or for some opcodes, two) sync waits.  Hoist multi-wait sync_info
    onto standalone same-engine EventSemaphore instructions inserted
    immediately before the instruction; program order on the engine's stream
    preserves semantics."""
    import orjson

    d = orjson.loads(bir_bytes)
    n = 0
    for fn in d["functions"]:
        for bb in fn["blocks"]:
            out = []
            for inst in bb["instructions"]:
                si = inst.get("sync_info")
                waits = (si or {}).get("on_wait") or []
                if len(waits) > 1:
                    for w in waits:
                        out.append(
                            {
                                "debug": inst.get("debug"),
                                "engine": inst["engine"],
                                "ins": [],
                                "name": f"I-waitfix-{n}",
                                "opcode": "EventSemaphore",
                                "outs": [],
                                "sync_info": {"on_wait": [w], "on_update": []},
                            }
                        )
                        n += 1
                    si["on_wait"] = []
                out.append(inst)
            bb["instructions"] = out
    return orjson.dumps(d)


def _install_bir_patch():
    if _cache.get("patched"):
        return
    import concourse.bass as bass

    orig = bass.Bass.to_json_bytes

    def patched(self):
        return _split_waits_bir(orig(self))

    bass.Bass.to_json_bytes = patched
    _cache["patched"] = True


def _build_nc():
    import concourse.bass as bass
    import concourse.tile as tile
    from concourse import mybir

    _install_bir_patch()

    f32 = mybir.dt.float32
    bf16 = mybir.dt.bfloat16
    add = mybir.AluOpType.add
    mult = mybir.AluOpType.mult

    nc = bass.Bass()
    xp = [
        nc.declare_dram_parameter(f"xp{r}", [P, NG], bf16, isOutput=False)
        for r in range(G)
    ]
    rrp = nc.declare_dram_parameter("rrp", [1, L], bf16, isOutput=False)
    outs = [
        nc.declare_dram_parameter(f"o{r}", [P, NG], bf16, isOutput=True)
        for r in range(G)
    ]

    with tile.TileContext(nc) as tc:
        with (
            tc.tile_pool(name="sb", bufs=1) as sb,
            tc.tile_pool(name="ps", bufs=1, space="PSUM") as ps,
        ):
            xs = [sb.tile([P, NG], bf16, tag=f"x{r}", name=f"x{r}") for r in range(G)]
            u01 = sb.tile([P, NG], bf16, tag="u01")
            v012 = sb.tile([P, NG], bf16, tag="v012")
            cumY = sb.tile([P, NG + 1], bf16, tag="cumY")
            cs = [sb.tile([P, NG], bf16, tag=f"c{r}", name=f"c{r}") for r in range(3)]
            os_ = [sb.tile([P, NG], bf16, tag=f"os{r}", name=f"os{r}") for r in range(G)]
            rr_sb = sb.tile([P, L], bf16, tag="rrsb")
            rrow_sb = sb.tile([1, L], bf16, tag="rrow")
            ones = sb.tile([1, P], bf16, tag="ones")

            # ---- input DMAs on two HW queues (aggregate BW is shared; the
            # split only parallelizes the ~630ns per-issue cost).
            # Balanced two-queue input (aggregate ~262 B/ns); the scan
            # gates on x3's semaphore alone (v012 covers x0+x1+x2), which
            # beats the old u23 pair-add whenever arrival phases skew.
            nc.sync.dma_start(rrow_sb[:], rrp[:])
            nc.sync.dma_start(xs[1][:], xp[1][:])
            nc.sync.dma_start(xs[3][:], xp[3][:])
            nc.scalar.dma_start(xs[0][:], xp[0][:])
            nc.scalar.dma_start(xs[2][:], xp[2][:])

            # ---- Pool: tiny memsets only (Pool tensor work stalls DVE 3x)
            nc.gpsimd.memset(ones[:], 1.0)
            nc.gpsimd.memset(cumY[:, 0:1], 0.0)

            # ---- PE: replicate rr across partitions; Act drains to bf16.
            # Segment 3 first: o3's mul is the first the wave runs.
            PB = 512
            pair_order = [3, 0, 1, 2]   # o3's rr segment first
            for pj in pair_order:
                pban = ps.tile([P, 2 * PB], f32, tag=f"rrp{pj}", name=f"rrp{pj}")
                for k in range(2):
                    j = 2 * pj + k
                    nc.tensor.matmul(
                        pban[:, k * PB : (k + 1) * PB],
                        ones[:],
                        rrow_sb[:, j * PB : (j + 1) * PB],
                        start=True,
                        stop=True,
                    )
                nc.scalar.copy(rr_sb[:, 2 * pj * PB : (2 * pj + 2) * PB], pban[:])

            def rr(r):
                return rr_sb[:, r * NG : (r + 1) * NG]

            # ---- DVE: pair adds (fp32 out: scan runs 2.3ns/col from fp32)
            nc.vector.tensor_tensor(u01[:], xs[0][:], xs[1][:], op=add)
            nc.vector.tensor_tensor(v012[:], u01[:], xs[2][:], op=add)

            # ---- DVE: group scan; consumes the last-arriving plane raw:
            # state = (v012[g] + state) + x3[g]
            nc.vector.tensor_tensor_scan(
                cumY[:, 1 : NG + 1], v012[:], xs[3][:], 0.0, op0=add, op1=add
            )
            carry = cumY[:, 0:NG]

            # ---- wave: c-chain with muls slotted into dependency bubbles
            h3 = NG // 2
            nc.vector.tensor_tensor(
                os_[3][:, 0:h3], cumY[:, 1 : h3 + 1],
                rr_sb[:, 3 * NG : 3 * NG + h3], op=mult,
            )
            nc.sync.dma_start(outs[3][:, 0:h3], os_[3][:, 0:h3])
            nc.vector.tensor_tensor(
                os_[3][:, h3:NG], cumY[:, h3 + 1 : NG + 1],
                rr_sb[:, 3 * NG + h3 : 4 * NG], op=mult,
            )
            nc.sync.dma_start(outs[3][:, h3:NG], os_[3][:, h3:NG])

            nc.vector.tensor_tensor(cs[0][:], xs[0][:], carry, op=add)
            for r in range(1, 3):
                nc.vector.tensor_tensor(os_[r - 1][:], cs[r - 1][:], rr(r - 1), op=mult)
                nc.vector.tensor_tensor(cs[r][:], cs[r - 1][:], xs[r][:], op=add)
                eng = nc.sync if r % 2 == 0 else nc.scalar
                eng.dma_start(outs[r - 1][:], os_[r - 1][:])
            h = NG // 2
            q = NG // 4
            nc.vector.tensor_tensor(os_[2][:, 0:h], cs[2][:, 0:h], rr_sb[:, 2 * NG : 2 * NG + h], op=mult)
            nc.scalar.dma_start(outs[2][:, 0:h], os_[2][:, 0:h])
            nc.vector.tensor_tensor(os_[2][:, h : h + q], cs[2][:, h : h + q], rr_sb[:, 2 * NG + h : 2 * NG + h + q], op=mult)
            nc.sync.dma_start(outs[2][:, h : h + q], os_[2][:, h : h + q])
            nc.vector.tensor_tensor(os_[2][:, h + q : NG], cs[2][:, h + q : NG], rr_sb[:, 2 * NG + h + q : 3 * NG], op=mult)
            nc.scalar.dma_start(outs[2][:, h + q : NG], os_[2][:, h + q : NG])
    return nc


def _get_nc():
    if "nc" not in _cache:
        _cache["nc"] = _build_nc()
    return _cache["nc"]


def _make_in_maps(x):
    import ml_dtypes

    bf16 = ml_dtypes.bfloat16
    idx = np.arange(1, L + 1, dtype=np.float64)
    rr_full = 1.0 / idx
    # plane-major rr row: rrp[0, r*NG + g] = 1/(4g + r + 1)
    rrp = np.empty((1, L), dtype=bf16)
    for r in range(G):
        rrp[0, r * NG : (r + 1) * NG] = rr_full[r::G].astype(bf16)
    in_maps = []
    shards = []
    for c in range(NCORES):
        b, dh = c // 2, c % 2
        shards.append((b, dh))
        xT = x[b, :, dh * P : (dh + 1) * P].T.astype(bf16)  # [128, 4096]
        m = {"rrp": rrp}
        for r in range(G):
            m[f"xp{r}"] = np.ascontiguousarray(xT[:, r::G])
        in_maps.append(m)
    return in_maps, shards


def kernel(x, q):
    from concourse.bass_utils import run_bass_kernel_spmd

    x = np.asarray(x)
    assert x.shape == (B, L, D) and x.dtype == np.float32

    nc = _get_nc()
    in_maps, shards = _make_in_maps(x)
    results = run_bass_kernel_spmd(nc, in_maps, list(range(NCORES))).results

    out = np.empty((B, L, D), dtype=np.float32)
    full = np.empty((P, L), dtype=np.float32)
    for c, (b, dh) in enumerate(shards):
        for r in range(G):
            full[:, r::G] = results[c][f"o{r}"].astype(np.float32)
        out[b, :, dh * P : (dh + 1) * P] = full.T
    return out


# revision 22
# speedup vs baseline: 1.1098x; 1.0196x over previous
"""Trainium2 Bass kernel for nn_CausalAttentionPooling.

Math: scores[b,i,j] = x[b,i].q are constant along the softmax axis j, so
softmax over the causal mask yields uniform weights 1/(i+1) on j <= i.
The module is exactly a causal cumulative mean:
    out[b,i,:] = cumsum(x, axis=1)[b,i,:] / (i+1)
(q does not affect the output.)

Sharding: 8 shards = (batch b in 0..3) x (D-half dh in 0..1); each core gets
x[b, :, dh*128:(dh+1)*128] transposed to [128(D), 4096(L)], cast bf16, and
de-interleaved on host into 4 planes xp[r] = xT[:, r::4] of [128, 1024].

Device algorithm (grouped cumsum, G=4):
  u01 = x0+x1 ; u23 = x2+x3                  (bf16 pair adds, DVE 2x mode)
  group scan absorbs the final pair add:
      state = (u01[g] + state) + u23[g]      (fp32 state) -> cumY bf16
  carry[g] = cumY[g-1]  (shifted view of a [128, NG+1] tile, col 0 = 0)
  c-chain: c0 = x0+carry; c1 = c0+x1; c2 = c1+x2
  o_r = c_r * rr_r.  o3 = cumY * rr_3 runs first, split in halves, so
  output DMAs hit the (idle) bus right after the scan; the last mul o2
  is split h/q/q so the final out-DMA is small.  The ~1MB of outputs
  needs ~4us of shared DMA bus: the tail is output-bandwidth-bound.
rr_r = 1/(4g+r+1) replicated across partitions via PE outer products
(idle engine), drained psum->bf16 SBUF by Act (idle engine).

Measured HW facts that shaped this (NTFF profiles, this chip):
  - ~9.5us fixed preamble+postamble (engine barriers, TENSOR_LOADs).
  - DMA aggregate ~262 B/ns per core SHARED across queues; two HW queues
    (SP, Act) only parallelize the ~630ns per-issue cost; completion
    semaphore posts ~0.9us after last byte.  Pool SWDGE queue is slower,
    not faster.  hwdge engines are only SP and Act.
  - DVE tensor_tensor bf16 SBUF runs 2x ((58+FD/2) cyc @0.96GHz); fp32
    or any-PSUM operand drops to 1x; tensor_tensor_scan is ~2.24ns/col
    regardless of src dtype; dependent-op cadence ~dur+215ns.
  - Per-op fixed cost (~130ns) favors FD=1024 ops: G=4 beats G=8.
  - Pool (gpsimd) tensor ops run ~2ns/elem AND stall concurrent DVE ops
    ~3x (SBUF contention) => Pool does only memsets.  Pool scan fails
    to compile.  Act has no tensor_tensor (activation-class ops only).
The spine (last input byte -> u23 -> scan) and the 7-op DVE wave are
engine/data-bound; ~25us total, vs 33.4/28.9us for the scan-everything
baseline and ~9.5us of it framework-fixed.
"""

import numpy as np

B, L, D = 4, 4096, 256
NCORES = 8
P = 128
G = 4
NG = L // G              # 1024 groups

_cache = {}


def _split_waits_bir(bir_bytes):
    """This container's walrus build rejects instructions carrying more than
    one (or for some opcodes, two) sync waits.  Hoist multi-wait sync_info
    onto standalone same-engine EventSemaphore instructions inserted
    immediately before the instruction; program order on the engine's stream
    preserves semantics."""
    import orjson

    d = orjson.loads(bir_bytes)
    n = 0
    for fn in d["functions"]:
        for bb in fn["blocks"]:
            out = []
            for inst in bb["instructions"]:
                si = inst.get("sync_info")
                waits = (si or {}).get("on_wait") or []
                if len(waits) > 1:
                    for w in waits:
                        out.append(
                            {
                                "debug": inst.get("debug"),
                                "engine": inst["engine"],
                                "ins": [],
                                "name": f"I-waitfix-{n}",
                                "opcode": "EventSemaphore",
                                "outs": [],
                                "sync_info": {"on_wait": [w], "on_update": []},
                            }
                        )
                        n += 1
                    si["on_wait"] = []
                out.append(inst)
            bb["instructions"] = out
    return orjson.dumps(d)


def _install_bir_patch():
    if _cache.get("patched"):
        return
    import concourse.bass as bass

    orig = bass.Bass.to_json_bytes

    def patched(self):
        return _split_waits_bir(orig(self))

    bass.Bass.to_json_bytes = patched
    _cache["patched"] = True


def _build_nc():
    import concourse.bass as bass
    import concourse.tile as tile
    from concourse import mybir

    _install_bir_patch()

    f32 = mybir.dt.float32
    bf16 = mybir.dt.bfloat16
    add = mybir.AluOpType.add
    mult = mybir.AluOpType.mult

    nc = bass.Bass()
    xp = [
        nc.declare_dram_parameter(f"xp{r}", [P, NG], bf16, isOutput=False)
        for r in range(G)
    ]
    rrp = nc.declare_dram_parameter("rrp", [1, L], bf16, isOutput=False)
    outs = [
        nc.declare_dram_parameter(f"o{r}", [P, NG], bf16, isOutput=True)
        for r in range(G)
    ]

    with tile.TileContext(nc) as tc:
        with (
            tc.tile_pool(name="sb", bufs=1) as sb,
            tc.tile_pool(name="ps", bufs=1, space="PSUM") as ps,
        ):
            xs = [sb.tile([P, NG], bf16, tag=f"x{r}", name=f"x{r}") for r in range(G)]
            u01 = sb.tile([P, NG], bf16, tag="u01")
            v012 = sb.tile([P, NG], bf16, tag="v012")
            cumY = sb.tile([P, NG + 1], bf16, tag="cumY")
            cs = [sb.tile([P, NG], bf16, tag=f"c{r}", name=f"c{r}") for r in range(3)]
            os_ = [sb.tile([P, NG], bf16, tag=f"os{r}", name=f"os{r}") for r in range(G)]
            rr_sb = sb.tile([P, L], bf16, tag="rrsb")
            rrow_sb = sb.tile([1, L], bf16, tag="rrow")
            ones = sb.tile([1, P], bf16, tag="ones")

            # ---- input DMAs on two HW queues (aggregate BW is shared; the
            # split only parallelizes the ~630ns per-issue cost).
            # Balanced two-queue input (aggregate ~262 B/ns); the scan
            # gates on x3's semaphore alone (v012 covers x0+x1+x2), which
            # beats the old u23 pair-add whenever arrival phases skew.
            nc.sync.dma_start(rrow_sb[:], rrp[:])
            nc.sync.dma_start(xs[1][:], xp[1][:])
            nc.sync.dma_start(xs[3][:], xp[3][:])
            nc.scalar.dma_start(xs[0][:], xp[0][:])
            nc.scalar.dma_start(xs[2][:], xp[2][:])

            # ---- Pool: tiny memsets only (Pool tensor work stalls DVE 3x)
            nc.gpsimd.memset(ones[:], 1.0)
            nc.gpsimd.memset(cumY[:, 0:1], 0.0)

            # ---- PE: replicate rr across partitions; Act drains to bf16.
            # Segment 3 first: o3's mul is the first the wave runs.
            PB = 512
            pair_order = [3, 0, 1, 2]   # o3's rr segment first
            for pj in pair_order:
                pban = ps.tile([P, 2 * PB], f32, tag=f"rrp{pj}", name=f"rrp{pj}")
                for k in range(2):
                    j = 2 * pj + k
                    nc.tensor.matmul(
                        pban[:, k * PB : (k + 1) * PB],
                        ones[:],
                        rrow_sb[:, j * PB : (j + 1) * PB],
                        start=True,
                        stop=True,
                    )
                nc.scalar.copy(rr_sb[:, 2 * pj * PB : (2 * pj + 2) * PB], pban[:])

            def rr(r):
                return rr_sb[:, r * NG : (r + 1) * NG]

            # ---- DVE: pair adds (fp32 out: scan runs 2.3ns/col from fp32)
            nc.vector.tensor_tensor(u01[:], xs[0][:], xs[1][:], op=add)
            nc.vector.tensor_tensor(v012[:], u01[:], xs[2][:], op=add)

            # ---- DVE: group scan; consumes the last-arriving plane raw:
            # state = (v012[g] + state) + x3[g]
            nc.vector.tensor_tensor_scan(
                cumY[:, 1 : NG + 1], v012[:], xs[3][:], 0.0, op0=add, op1=add
            )
            carry = cumY[:, 0:NG]

            # ---- wave: c-chain with muls slotted into dependency bubbles
            h3 = NG // 2
            nc.vector.tensor_tensor(
                os_[3][:, 0:h3], cumY[:, 1 : h3 + 1],
                rr_sb[:, 3 * NG : 3 * NG + h3], op=mult,
            )
            nc.sync.dma_start(outs[3][:, 0:h3], os_[3][:, 0:h3])
            nc.vector.tensor_tensor(
                os_[3][:, h3:NG], cumY[:, h3 + 1 : NG + 1],
                rr_sb[:, 3 * NG + h3 : 4 * NG], op=mult,
            )
            nc.sync.dma_start(outs[3][:, h3:NG], os_[3][:, h3:NG])

            # independent c-adds (u01/v012 are the in-group prefixes):
            # depth 1 after the scan instead of a 3-link chain
            csrc = [xs[0], u01, v012]
            nc.vector.tensor_tensor(cs[0][:], csrc[0][:], carry, op=add)
            for r in range(1, 3):
                nc.vector.tensor_tensor(os_[r - 1][:], cs[r - 1][:], rr(r - 1), op=mult)
                nc.vector.tensor_tensor(cs[r][:], csrc[r][:], carry, op=add)
                eng = nc.sync if r % 2 == 0 else nc.scalar
                eng.dma_start(outs[r - 1][:], os_[r - 1][:])
            h = NG // 2
            q = NG // 4
            nc.vector.tensor_tensor(os_[2][:, 0:h], cs[2][:, 0:h], rr_sb[:, 2 * NG : 2 * NG + h], op=mult)
            nc.scalar.dma_start(outs[2][:, 0:h], os_[2][:, 0:h])
            nc.vector.tensor_tensor(os_[2][:, h : h + q], cs[2][:, h : h + q], rr_sb[:, 2 * NG + h : 2 * NG + h + q], op=mult)
            nc.sync.dma_start(outs[2][:, h : h + q], os_[2][:, h : h + q])
            nc.vector.tensor_tensor(os_[2][:, h + q : NG], cs[2][:, h + q : NG], rr_sb[:, 2 * NG + h + q : 3 * NG], op=mult)
            nc.scalar.dma_start(outs[2][:, h + q : NG], os_[2][:, h + q : NG])
    return nc


def _get_nc():
    if "nc" not in _cache:
        _cache["nc"] = _build_nc()
    return _cache["nc"]


def _make_in_maps(x):
    import ml_dtypes

    bf16 = ml_dtypes.bfloat16
    idx = np.arange(1, L + 1, dtype=np.float64)
    rr_full = 1.0 / idx
    # plane-major rr row: rrp[0, r*NG + g] = 1/(4g + r + 1)
    rrp = np.empty((1, L), dtype=bf16)
    for r in range(G):
        rrp[0, r * NG : (r + 1) * NG] = rr_full[r::G].astype(bf16)
    in_maps = []
    shards = []
    for c in range(NCORES):
        b, dh = c // 2, c % 2
        shards.append((b, dh))
        xT = x[b, :, dh * P : (dh + 1) * P].T.astype(bf16)  # [128, 4096]
        m = {"rrp": rrp}
        for r in range(G):
            m[f"xp{r}"] = np.ascontiguousarray(xT[:, r::G])
        in_maps.append(m)
    return in_maps, shards


def kernel(x, q):
    from concourse.bass_utils import run_bass_kernel_spmd

    x = np.asarray(x)
    assert x.shape == (B, L, D) and x.dtype == np.float32

    nc = _get_nc()
    in_maps, shards = _make_in_maps(x)
    results = run_bass_kernel_spmd(nc, in_maps, list(range(NCORES))).results

    out = np.empty((B, L, D), dtype=np.float32)
    full = np.empty((P, L), dtype=np.float32)
    for c, (b, dh) in enumerate(shards):
        for r in range(G):
            full[:, r::G] = results[c][f"o{r}"].astype(np.float32)
        out[b, :, dh * P : (dh + 1) * P] = full.T
    return out


# revision 23
# speedup vs baseline: 1.1231x; 1.0119x over previous
"""Trainium2 Bass kernel for nn_CausalAttentionPooling.

Math: scores[b,i,j] = x[b,i].q are constant along the softmax axis j, so
softmax over the causal mask yields uniform weights 1/(i+1) on j <= i.
The module is exactly a causal cumulative mean:
    out[b,i,:] = cumsum(x, axis=1)[b,i,:] / (i+1)
(q does not affect the output.)

Sharding: 8 shards = (batch b in 0..3) x (D-half dh in 0..1); each core gets
x[b, :, dh*128:(dh+1)*128] transposed to [128(D), 4096(L)], cast bf16, and
de-interleaved on host into 4 planes xp[r] = xT[:, r::4] of [128, 1024].

Device algorithm (grouped cumsum, G=4):
  u01 = x0+x1 ; u23 = x2+x3                  (bf16 pair adds, DVE 2x mode)
  group scan absorbs the final pair add:
      state = (u01[g] + state) + u23[g]      (fp32 state) -> cumY bf16
  carry[g] = cumY[g-1]  (shifted view of a [128, NG+1] tile, col 0 = 0)
  c-chain: c0 = x0+carry; c1 = c0+x1; c2 = c1+x2
  o_r = c_r * rr_r.  o3 = cumY * rr_3 runs first, split in halves, so
  output DMAs hit the (idle) bus right after the scan; the last mul o2
  is split h/q/q so the final out-DMA is small.  The ~1MB of outputs
  needs ~4us of shared DMA bus: the tail is output-bandwidth-bound.
rr_r = 1/(4g+r+1) replicated across partitions via PE outer products
(idle engine), drained psum->bf16 SBUF by Act (idle engine).

Measured HW facts that shaped this (NTFF profiles, this chip):
  - ~9.5us fixed preamble+postamble (engine barriers, TENSOR_LOADs).
  - DMA aggregate ~262 B/ns per core SHARED across queues; two HW queues
    (SP, Act) only parallelize the ~630ns per-issue cost; completion
    semaphore posts ~0.9us after last byte.  Pool SWDGE queue is slower,
    not faster.  hwdge engines are only SP and Act.
  - DVE tensor_tensor bf16 SBUF runs 2x ((58+FD/2) cyc @0.96GHz); fp32
    or any-PSUM operand drops to 1x; tensor_tensor_scan is ~2.24ns/col
    regardless of src dtype; dependent-op cadence ~dur+215ns.
  - Per-op fixed cost (~130ns) favors FD=1024 ops: G=4 beats G=8.
  - Pool (gpsimd) tensor ops run ~2ns/elem AND stall concurrent DVE ops
    ~3x (SBUF contention) => Pool does only memsets.  Pool scan fails
    to compile.  Act has no tensor_tensor (activation-class ops only).
The spine (last input byte -> u23 -> scan) and the 7-op DVE wave are
engine/data-bound; ~25us total, vs 33.4/28.9us for the scan-everything
baseline and ~9.5us of it framework-fixed.
"""

import numpy as np

B, L, D = 4, 4096, 256
NCORES = 8
P = 128
G = 4
NG = L // G              # 1024 groups

_cache = {}


def _split_waits_bir(bir_bytes):
    """This container's walrus build rejects instructions carrying more than
    one (or for some opcodes, two) sync waits.  Hoist multi-wait sync_info
    onto standalone same-engine EventSemaphore instructions inserted
    immediately before the instruction; program order on the engine's stream
    preserves semantics."""
    import orjson

    d = orjson.loads(bir_bytes)
    n = 0
    for fn in d["functions"]:
        for bb in fn["blocks"]:
            out = []
            for inst in bb["instructions"]:
                si = inst.get("sync_info")
                waits = (si or {}).get("on_wait") or []
                if len(waits) > 1:
                    for w in waits:
                        out.append(
                            {
                                "debug": inst.get("debug"),
                                "engine": inst["engine"],
                                "ins": [],
                                "name": f"I-waitfix-{n}",
                                "opcode": "EventSemaphore",
                                "outs": [],
                                "sync_info": {"on_wait": [w], "on_update": []},
                            }
                        )
                        n += 1
                    si["on_wait"] = []
                out.append(inst)
            bb["instructions"] = out
    return orjson.dumps(d)


def _install_bir_patch():
    if _cache.get("patched"):
        return
    import concourse.bass as bass

    orig = bass.Bass.to_json_bytes

    def patched(self):
        return _split_waits_bir(orig(self))

    bass.Bass.to_json_bytes = patched
    _cache["patched"] = True


def _build_nc():
    import concourse.bass as bass
    import concourse.tile as tile
    from concourse import mybir

    _install_bir_patch()

    f32 = mybir.dt.float32
    bf16 = mybir.dt.bfloat16
    add = mybir.AluOpType.add
    mult = mybir.AluOpType.mult

    nc = bass.Bass()
    xp = [
        nc.declare_dram_parameter(f"xp{r}", [P, NG], bf16, isOutput=False)
        for r in range(G)
    ]
    rrp = nc.declare_dram_parameter("rrp", [1, L], bf16, isOutput=False)
    outs = [
        nc.declare_dram_parameter(f"o{r}", [P, NG], bf16, isOutput=True)
        for r in range(G)
    ]

    with tile.TileContext(nc) as tc:
        with (
            tc.tile_pool(name="sb", bufs=1) as sb,
            tc.tile_pool(name="ps", bufs=1, space="PSUM") as ps,
        ):
            xs = [sb.tile([P, NG], bf16, tag=f"x{r}", name=f"x{r}") for r in range(G)]
            u01 = sb.tile([P, NG], bf16, tag="u01")
            v012 = sb.tile([P, NG], bf16, tag="v012")
            cumY = sb.tile([P, NG + 1], bf16, tag="cumY")
            cs = [sb.tile([P, NG], bf16, tag=f"c{r}", name=f"c{r}") for r in range(3)]
            os_ = [sb.tile([P, NG], bf16, tag=f"os{r}", name=f"os{r}") for r in range(G)]
            rr_sb = sb.tile([P, L], bf16, tag="rrsb")
            rrow_sb = sb.tile([1, L], bf16, tag="rrow")
            ones = sb.tile([1, P], bf16, tag="ones")

            # ---- input DMAs on two HW queues (aggregate BW is shared; the
            # split only parallelizes the ~630ns per-issue cost).
            # Balanced two-queue input (aggregate ~262 B/ns); the scan
            # gates on x3's semaphore alone (v012 covers x0+x1+x2), which
            # beats the old u23 pair-add whenever arrival phases skew.
            nc.sync.dma_start(rrow_sb[:], rrp[:])
            nc.sync.dma_start(xs[1][:], xp[1][:])
            nc.sync.dma_start(xs[3][:], xp[3][:])
            nc.scalar.dma_start(xs[0][:], xp[0][:])
            nc.scalar.dma_start(xs[2][:], xp[2][:])

            # ---- Pool: tiny memsets only (Pool tensor work stalls DVE 3x)
            nc.gpsimd.memset(ones[:], 1.0)
            nc.gpsimd.memset(cumY[:, 0:1], 0.0)

            # ---- PE: replicate rr across partitions; Act drains to bf16.
            # Segment 3 first: o3's mul is the first the wave runs.
            PB = 512
            pair_order = [3, 0, 1, 2]   # o3's rr segment first
            for pj in pair_order:
                pban = ps.tile([P, 2 * PB], f32, tag=f"rrp{pj}", name=f"rrp{pj}")
                for k in range(2):
                    j = 2 * pj + k
                    nc.tensor.matmul(
                        pban[:, k * PB : (k + 1) * PB],
                        ones[:],
                        rrow_sb[:, j * PB : (j + 1) * PB],
                        start=True,
                        stop=True,
                    )
                nc.scalar.copy(rr_sb[:, 2 * pj * PB : (2 * pj + 2) * PB], pban[:])

            def rr(r):
                return rr_sb[:, r * NG : (r + 1) * NG]

            # ---- DVE: pair adds (fp32 out: scan runs 2.3ns/col from fp32)
            nc.vector.tensor_tensor(u01[:], xs[0][:], xs[1][:], op=add)
            nc.vector.tensor_tensor(v012[:], u01[:], xs[2][:], op=add)

            # ---- DVE: group scan; consumes the last-arriving plane raw:
            # state = (v012[g] + state) + x3[g]
            nc.vector.tensor_tensor_scan(
                cumY[:, 1 : NG + 1], v012[:], xs[3][:], 0.0, op0=add, op1=add
            )
            carry = cumY[:, 0:NG]

            # ---- wave: c-chain with muls slotted into dependency bubbles
            # o3 unsplit: tail is last-mul-gated, so one FD=1024 mul
            # (692ns) beats two FD=512 halves (834ns); o3's transfer is
            # early enough that the bus absorbs it either way.
            nc.vector.tensor_tensor(os_[3][:], cumY[:, 1 : NG + 1], rr(3), op=mult)
            nc.sync.dma_start(outs[3][:], os_[3][:])

            # independent c-adds (u01/v012 are the in-group prefixes):
            # depth 1 after the scan instead of a 3-link chain
            csrc = [xs[0], u01, v012]
            nc.vector.tensor_tensor(cs[0][:], csrc[0][:], carry, op=add)
            for r in range(1, 3):
                nc.vector.tensor_tensor(os_[r - 1][:], cs[r - 1][:], rr(r - 1), op=mult)
                nc.vector.tensor_tensor(cs[r][:], csrc[r][:], carry, op=add)
                eng = nc.sync if r % 2 == 0 else nc.scalar
                eng.dma_start(outs[r - 1][:], os_[r - 1][:])
            h = NG // 2
            q = NG // 4
            nc.vector.tensor_tensor(os_[2][:, 0:h], cs[2][:, 0:h], rr_sb[:, 2 * NG : 2 * NG + h], op=mult)
            nc.scalar.dma_start(outs[2][:, 0:h], os_[2][:, 0:h])
            nc.vector.tensor_tensor(os_[2][:, h : h + q], cs[2][:, h : h + q], rr_sb[:, 2 * NG + h : 2 * NG + h + q], op=mult)
            nc.sync.dma_start(outs[2][:, h : h + q], os_[2][:, h : h + q])
            nc.vector.tensor_tensor(os_[2][:, h + q : NG], cs[2][:, h + q : NG], rr_sb[:, 2 * NG + h + q : 3 * NG], op=mult)
            nc.scalar.dma_start(outs[2][:, h + q : NG], os_[2][:, h + q : NG])
    return nc


def _get_nc():
    if "nc" not in _cache:
        _cache["nc"] = _build_nc()
    return _cache["nc"]


def _make_in_maps(x):
    import ml_dtypes

    bf16 = ml_dtypes.bfloat16
    idx = np.arange(1, L + 1, dtype=np.float64)
    rr_full = 1.0 / idx
    # plane-major rr row: rrp[0, r*NG + g] = 1/(4g + r + 1)
    rrp = np.empty((1, L), dtype=bf16)
    for r in range(G):
        rrp[0, r * NG : (r + 1) * NG] = rr_full[r::G].astype(bf16)
    in_maps = []
    shards = []
    for c in range(NCORES):
        b, dh = c // 2, c % 2
        shards.append((b, dh))
        xT = x[b, :, dh * P : (dh + 1) * P].T.astype(bf16)  # [128, 4096]
        m = {"rrp": rrp}
        for r in range(G):
            m[f"xp{r}"] = np.ascontiguousarray(xT[:, r::G])
        in_maps.append(m)
    return in_maps, shards


def kernel(x, q):
    from concourse.bass_utils import run_bass_kernel_spmd

    x = np.asarray(x)
    assert x.shape == (B, L, D) and x.dtype == np.float32

    nc = _get_nc()
    in_maps, shards = _make_in_maps(x)
    results = run_bass_kernel_spmd(nc, in_maps, list(range(NCORES))).results

    out = np.empty((B, L, D), dtype=np.float32)
    full = np.empty((P, L), dtype=np.float32)
    for c, (b, dh) in enumerate(shards):
        for r in range(G):
            full[:, r::G] = results[c][f"o{r}"].astype(np.float32)
        out[b, :, dh * P : (dh + 1) * P] = full.T
    return out
